# revision 61
# baseline (speedup 1.0000x reference)
"""Trainium2 Bass kernel for a pre-norm transformer block (attention + MLP).

Problem: x:[2, 2048, 1024], 16 heads x 64, MLP hidden 4096, fp32.

Sharding: data parallel over tokens, zero collectives. The 4096 tokens are
split into 8 blocks of 512 (core c handles batch c//4, sequence block c%4).
Each core receives its batch's 2048 rows ROTATED so its own 512 tokens are
rows 0:512 (keeps the program SPMD-uniform); it recomputes K/V for the whole
batch, runs attention for its 512 queries over all 2048 keys, then the MLP
for its own tokens. The host reassembles the output from 8 [512, 1024]
blocks.

Kernel structure (per core):
  - Fused phase 1: per 128-token block, LayerNorm stats (DVE bn_stats), the
    normalize runs on the Scalar/Act engine (per-partition scale=rsig,
    bias=-mu*rsig) emitting bf16, PE-transposes the block to channel-major
    (bf16, 1 cyc/row), drains to fp8e4, and immediately projects that
    block's K and V columns (all 16 heads) plus Q for own blocks.
  - Projections (q/k/v and attn-out) run in fp8e4 with DoubleRow perf mode
    (2 channel-blocks contracted per pass, 0.5 cyc/row): the accumulation
    over channel-block pairs maps directly onto DoubleRow's [p, 2, f]
    operand layout, so it is a pure reinterpretation of the bf16 tiling.
    Weights are pre-scaled x16 on the host (fp8e4 subnormal avoidance) and
    descaled in the PSUM drains. fc1/fc2 stay bf16: their error feeds the
    residual stream directly and fp8 there would blow the 2e-2 budget,
    whereas q/k errors cancel in softmax renormalization and v/proj errors
    are damped by the diffuse attention average (n_eff ~ 570) and the small
    attention-output magnitude (rms 0.064).
  - Attention stays bf16: scores transposed S^T[m, t] = k^T.T @ q^T with two
    row-packed K=64 matmuls per head pair; exp on the Act engine; V carries
    a ones column so the attention-value matmul accumulates softmax
    denominators; the 1/sum normalization is applied to the 64-row head
    outputs (scaled x16 into fp8 oT for the DoubleRow attn projection).
  - MLP: fc1+gelu into gT, fc2 with 8 held PSUM accumulators + residual.
"""

import numpy as np
from contextlib import ExitStack

import concourse.bass as bass
import concourse.tile as tile
from concourse import mybir
from concourse.bass_utils import run_bass_kernel_spmd
from concourse.masks import make_identity

FP32 = mybir.dt.float32
BF16 = mybir.dt.bfloat16
FP8 = mybir.dt.float8e4
AF = mybir.ActivationFunctionType
ALU = mybir.AluOpType
DR = mybir.MatmulPerfMode.DoubleRow

N_CORES = 8
B, N, C, H, D, F = 2, 2048, 1024, 16, 64, 4096
T = 512            # tokens owned per core
M = 2048           # keys (full batch sequence)
EPS = 1e-5
SCALE = float(D) ** -0.5   # 0.125
WS = 16.0          # fp8 weight pre-scale
IWS = 1.0 / WS

CB = C // 128      # 8 channel blocks
TB = T // 128      # 4 own-token blocks
MI = M // 128      # 16 key 128-chunks
FB = F // 128      # 32 mlp hidden blocks

SAB_BUFS = 2
STOP_AFTER = ""    # debug: stop emission after a phase name
K_ENG = "v"        # k-drain engine: v=DVE, p=Pool, a=Act
V_ENG = "a"        # v-drain engine
Q_ENG = "a"        # q-drain engine
SC_ENG = "a"       # proj descale engine
W1_BUFS = 4
P1H_BUFS = 3


def _drain(nc, eng, out, in_, scale, bias_ap=None):
    """PSUM->SBUF drain with scale (+ optional per-partition bias) on a
    selectable engine: v=DVE, p=Pool/gpsimd, a=Scalar/Act."""
    if eng == "a":
        if bias_ap is None:
            nc.scalar.activation(out=out, in_=in_, func=AF.Copy, scale=scale)
        else:
            nc.scalar.activation(out=out, in_=in_, func=AF.Identity,
                                 bias=bias_ap, scale=scale)
    else:
        e = nc.vector if eng == "v" else nc.gpsimd
        if bias_ap is None:
            e.tensor_scalar(out=out, in0=in_, scalar1=scale, scalar2=None,
                            op0=ALU.mult)
        else:
            e.tensor_scalar(out=out, in0=in_, scalar1=scale, scalar2=bias_ap,
                            op0=ALU.mult, op1=ALU.add)


def _ln_stats(nc, work, x_sb, eps_sb):
    """rsig [128,1] and negmurs=-mu*rsig [128,1] for LN along free dim."""
    stats = work.tile([128, 2, 6], FP32, name="ln_stats")
    nc.vector.bn_stats(out=stats[:, 0, :], in_=x_sb[:, 0:512])
    nc.vector.bn_stats(out=stats[:, 1, :], in_=x_sb[:, 512:1024])
    mv = work.tile([128, 2], FP32, name="ln_mv")
    nc.vector.bn_aggr(out=mv, in_=stats)
    sd = work.tile([128, 1], FP32, name="ln_sd")
    nc.scalar.activation(out=sd, in_=mv[:, 1:2], func=AF.Sqrt, bias=eps_sb,
                         scale=1.0)
    rsig = work.tile([128, 1], FP32, name="ln_rsig")
    nc.vector.reciprocal(out=rsig, in_=sd)
    murs = work.tile([128, 1], FP32, name="ln_murs")
    nc.vector.tensor_mul(out=murs, in0=mv[:, 0:1], in1=rsig)
    negmurs = work.tile([128, 1], FP32, name="ln_negmurs")
    nc.vector.tensor_scalar(out=negmurs, in0=murs, scalar1=-1.0, scalar2=None,
                            op0=ALU.mult)
    return rsig, negmurs


def _emit(ctx: ExitStack, tc: tile.TileContext, io: dict):
    nc = tc.nc

    xb = io["xb"]          # [2048, 1024] rotated batch rows (own = 0:512)
    qkv_w = io["qkv_w"]    # [1024, 3072] fp8 (ln1_w folded, x16)
    proj_w = io["proj_w"]  # [1024, 1024] fp8 (x16)
    proj_b = io["proj_b"]  # [1024] fp32 (+ folded v bias)
    q_bias = io["q_bias"]  # [1024] folded ln1_b @ Wq
    fc1_w, fc1_b = io["fc1_w"], io["fc1_b"]   # bf16 / fp32 (ln2 folded)
    fc2_w, fc2_b = io["fc2_w"], io["fc2_b"]
    y = io["y"]            # [512, 1024] output

    xb_r = xb.rearrange("(mi p) c -> mi p c", p=128)          # [16, 128, 1024]
    xown_r = xb.rearrange("(tb p) c -> p tb c", p=128)        # view; tb<4 own
    qkv_r = qkv_w.rearrange("(cb p) o -> p cb o", p=128)      # [128, 8, 3072]
    proj_r = proj_w.rearrange("(cb p) o -> p cb o", p=128)    # [128, 8, 1024]
    fc1_r = fc1_w.rearrange("(cb p) f -> p cb f", p=128)      # [128, 8, 4096]
    fc2_r = fc2_w.rearrange("(fb p) c -> p fb c", p=128)      # [128, 32, 1024]
    y_r = y.rearrange("(tb p) c -> p tb c", p=128)            # [128, 4, 1024]

    # --- constants (live whole kernel) ---
    consts = ctx.enter_context(tc.tile_pool(name="consts", bufs=1))

    ident_f = consts.tile([128, 128], FP32)
    make_identity(nc, ident_f)
    ident = consts.tile([128, 128], BF16)
    nc.vector.tensor_copy(out=ident, in_=ident_f)
    ones_f = consts.tile([128, 128], FP32)
    nc.vector.memset(ones_f, 1.0)
    ones = consts.tile([128, 128], BF16)
    nc.vector.tensor_copy(out=ones, in_=ones_f)
    sixt_f = consts.tile([128, 64], FP32)
    nc.vector.memset(sixt_f, WS)
    sixt = consts.tile([128, 64], BF16)
    nc.vector.tensor_copy(out=sixt, in_=sixt_f)
    eps_sb = consts.tile([128, 1], FP32)
    nc.vector.memset(eps_sb, EPS)

    def load_vec_pcb(vec, nblk, name):
        t = consts.tile([128, nblk], FP32, name=name)
        nc.sync.dma_start(out=t, in_=vec.rearrange("(b p) -> p b", p=128))
        return t

    def bcast_rows_pool(pool, vec, name):
        t = pool.tile([128, C], FP32, name=name)
        src = bass.AP(tensor=vec.tensor, offset=vec.offset,
                      ap=[[0, 128]] + vec.ap)
        nc.sync.dma_start(out=t, in_=src)
        return t

    # --- mid tensors (whole kernel) + attention operands (phases 1-3) ---
    p_mid = ctx.enter_context(tc.tile_pool(name="p_mid", bufs=1))
    x2 = p_mid.tile([128, TB, C], FP32)          # residual after attention
    h2T = p_mid.tile([128, CB, T], BF16)         # LN2 out, channel-major

    # p_big holds the attention operands; it is closed (LIFO) after the
    # attn-projection phase so the MLP can stage fc2 weights in its space
    pbig_stack = ExitStack()
    p_big = pbig_stack.enter_context(tc.tile_pool(name="p_big", bufs=1))
    kT = p_big.tile([128, CB, M], BF16)          # k channel-major, 16 heads
    vg = p_big.tile([128, MI, H, D + 1], BF16)   # v token-major + ones col
    qT = p_big.tile([128, CB, T], BF16)          # q channel-major (own)
    oT = p_big.tile([128, CB, T], FP8)           # attn out x16, fp8

    # ones column of vg (all mi, all heads at free-offset D)
    ones_col = bass.AP(
        tensor=ones.tensor, offset=ones[:, 0:1].offset,
        ap=[ones.ap[0], [0, MI], [0, H], [1, 1]])
    nc.vector.tensor_copy(out=vg[:, :, :, D:D + 1], in_=ones_col)

    # ------------------------------------------------------------------
    # Phase 1 (fused): per 128-token block: LN -> transpose -> K/V (+Q own)
    # ------------------------------------------------------------------
    with (
        tc.tile_pool(name="p1_w", bufs=1) as p1w,
        tc.tile_pool(name="p1_work", bufs=W1_BUFS) as w1,
        tc.tile_pool(name="p1_h", bufs=P1H_BUFS) as p1h,
        tc.tile_pool(name="p1_pst", bufs=2, space="PSUM") as ps_t,
        tc.tile_pool(name="p1_psp", bufs=2, space="PSUM") as ps_p,
    ):
        # first x blocks before the (bigger) weight DMAs so LN starts early
        xc_pre = []
        for mi in range(3):
            xc = w1.tile([128, C], FP32, name="p1_x")
            nc.sync.dma_start(out=xc, in_=xb_r[mi])
            xc_pre.append(xc)
        wq8 = p1w.tile([128, CB, C], FP8, name="wq8")
        nc.sync.dma_start(out=wq8, in_=qkv_r[:, :, 0:C])
        wk8 = p1w.tile([128, CB, C], FP8, name="wk8")
        nc.sync.dma_start(out=wk8, in_=qkv_r[:, :, C:2 * C])
        wv8 = p1w.tile([128, CB, C], FP8, name="wv8")
        nc.sync.dma_start(out=wv8, in_=qkv_r[:, :, 2 * C:3 * C])
        qb_sb = load_vec_pcb(q_bias, CB, "qb")
        fc1b_sb = load_vec_pcb(fc1_b, FB, "fc1b")

        def emit_ln(mi):
            if mi < 3:
                xc = xc_pre[mi]
            else:
                xc = w1.tile([128, C], FP32, name="p1_x")
                nc.sync.dma_start(out=xc, in_=xb_r[mi])
            rsig, negmurs = _ln_stats(nc, w1, xc, eps_sb)
            xn = w1.tile([128, C], BF16, name="p1_xn")
            nc.scalar.activation(out=xn, in_=xc, func=AF.Identity,
                                 bias=negmurs, scale=rsig)
            hT8 = p1h.tile([128, CB, 128], FP8, name="hT8")
            for h4 in range(2):
                tp = ps_t.tile([128, 4, 128], BF16, name="p1_tp")
                for j in range(4):
                    cb = h4 * 4 + j
                    nc.tensor.transpose(
                        tp[:, j, :], xn[:, cb * 128:(cb + 1) * 128], ident)
                nc.vector.tensor_copy(
                    out=hT8[:, h4 * 4:(h4 + 1) * 4, :], in_=tp)
            return hT8

        # transposes run one block ahead of the projections so the PE
        # never waits on the Pool-engine hT8 drain
        hq = [emit_ln(0)]
        for mi in range(MI):
            if mi + 1 < MI:
                hq.append(emit_ln(mi + 1))
            hT8 = hq.pop(0)

            # K chunk: all 16 heads for this block, channel-major
            for half in range(2):
                pk = ps_p.tile([128, 4, 128], FP32, name="p1_pk")
                for kb4 in range(4):
                    kb = half * 4 + kb4
                    for j in range(4):
                        nc.tensor.matmul(
                            pk[:, kb4, :],
                            wk8[:, 2 * j:2 * j + 2, kb * 128:(kb + 1) * 128],
                            hT8[:, 2 * j:2 * j + 2, :],
                            start=(j == 0), stop=(j == 3), perf_mode=DR)
                kslc = slice(half * 4, (half + 1) * 4)
                _drain(nc, K_ENG, kT[:, kslc, mi * 128:(mi + 1) * 128],
                       pk, IWS)

            # V chunk: token-major [block, 16 heads x 64], + descale
            for half in range(2):
                pv = ps_p.tile([128, 512], FP32, name="p1_pv")
                for j in range(4):
                    nc.tensor.matmul(
                        pv, hT8[:, 2 * j:2 * j + 2, :],
                        wv8[:, 2 * j:2 * j + 2,
                            half * 512:(half + 1) * 512],
                        start=(j == 0), stop=(j == 3), perf_mode=DR)
                _drain(nc, V_ENG, vg[:, mi, half * 8:(half + 1) * 8, 0:D],
                       pv.rearrange("p (h d) -> p h d", d=D), IWS)

            # Q for own blocks (rotated: always blocks 0..3)
            if mi < TB:
                for half in range(2):
                    pq = ps_p.tile([128, 4, 128], FP32, name="p1_pq")
                    for qb4 in range(4):
                        qb = half * 4 + qb4
                        for j in range(4):
                            nc.tensor.matmul(
                                pq[:, qb4, :],
                                wq8[:, 2 * j:2 * j + 2,
                                    qb * 128:(qb + 1) * 128],
                                hT8[:, 2 * j:2 * j + 2, :],
                                start=(j == 0), stop=(j == 3), perf_mode=DR)
                    for qb4 in range(4):
                        qb = half * 4 + qb4
                        _drain(nc, Q_ENG,
                               qT[:, qb, mi * 128:(mi + 1) * 128],
                               pq[:, qb4, :], IWS, qb_sb[:, qb:qb + 1])

    if STOP_AFTER == "ln1":
        return

    # ------------------------------------------------------------------
    # Phase 2: attention, 8 head pairs, scores over 16 key chunks.
    # Unit = (pair, mi, head): one score matmul -> exp -> one AV matmul.
    # Scores are emitted 2 units ahead of the AV consumer so the Act
    # engine's exp latency never stalls the PE.
    # ------------------------------------------------------------------
    # prefetch the proj-phase operands so their DMAs run under attention
    wpj = p_mid.tile([128, CB, C], FP8, name="wpj")
    nc.sync.dma_start(out=wpj, in_=proj_r)
    b1bc = bcast_rows_pool(p_mid, proj_b, "b1bc")
    xob = p_mid.tile([128, TB, C], FP32, name="xob")
    nc.sync.dma_start(out=xob, in_=xown_r[:, 0:TB, :])
    for tb in range(TB):
        nc.gpsimd.tensor_add(out=xob[:, tb, :], in0=xob[:, tb, :],
                             in1=b1bc)

    with (
        tc.tile_pool(name="a_p", bufs=3) as pp,
        tc.tile_pool(name="a_r", bufs=2) as pr,
        tc.tile_pool(name="a_dram", bufs=2, space="DRAM") as p_dram,
        tc.tile_pool(name="a_pss", bufs=3, space="PSUM") as ps_s,
        tc.tile_pool(name="a_pso", bufs=1, space="PSUM") as ps_o,
    ):
        NU = (H // 2) * MI     # units: (pair, mi)

        def emit_scores(u):
            pair, mi = divmod(u, MI)
            msl = slice(mi * 128, (mi + 1) * 128)
            sAB = ps_s.tile([128, 2, T], FP32, name="sAB")
            nc.tensor.matmul(
                sAB[:, 0, :], kT[0:64, pair, msl],
                qT[0:64, pair, :], start=True, stop=True)
            nc.tensor.matmul(
                sAB[:, 1, :], kT[64:128, pair, msl],
                qT[64:128, pair, :], start=True, stop=True)
            return sAB

        def norm_head(o_sb, hh, pair, fast):
            """o_sb: [65, T] SBUF copy (row 64 = softmax sums). The 16/sum
            row is partition-broadcast either via a DRAM bounce (cheap on
            engines, ~2 DMA latencies — fine mid-attention where the tail
            hides under the next pair) or, for the LAST pair (critical path
            into the proj phase), via a PE ones-matmul into a free
            score-PSUM slot (x16 folded into the sixt constant)."""
            rec = pr.tile([128, T], FP32, name=f"rec{hh}")
            nc.vector.reciprocal(out=rec[64:65, :], in_=o_sb[64:65, :])
            if fast:
                recr = pr.tile([128, T], BF16, name=f"recr{hh}")
                nc.vector.tensor_copy(out=recr[64:65, :], in_=rec[64:65, :])
                rb_ps = ps_s.tile([128, 2, T], FP32, name="sAB")[:, 0, :]
                nc.tensor.matmul(
                    rb_ps[0:64, :], sixt[64:65, 0:64], recr[64:65, :],
                    start=True, stop=True)
                rb = rb_ps
            else:
                nc.vector.tensor_scalar(
                    out=rec[64:65, :], in0=rec[64:65, :], scalar1=WS,
                    scalar2=None, op0=ALU.mult)
                dr = p_dram.tile([T], FP32, name=f"dr{hh}")
                nc.sync.dma_start(out=dr, in_=rec[64:65, :])
                rb = pr.tile([128, T], FP32, name=f"rb{hh}")
                src = bass.AP(tensor=dr.tensor, offset=dr.offset,
                              ap=[[0, 64]] + dr.ap)
                nc.sync.dma_start(out=rb[0:64, :], in_=src)
            if hh == 0:
                nc.vector.tensor_mul(
                    out=oT[0:64, pair, :], in0=o_sb[0:64, :],
                    in1=rb[0:64, :])
            else:
                tmpB = pr.tile([128, T], FP8, name="tmpB")
                nc.vector.tensor_mul(
                    out=tmpB[0:64, :], in0=o_sb[0:64, :], in1=rb[0:64, :])
                nc.sync.dma_start(out=oT[64:128, pair, :], in_=tmpB[0:64, :])

        DEPTH = 2
        sq = [emit_scores(u) for u in range(DEPTH)]
        for u in range(NU):
            pair, mi = divmod(u, MI)
            if mi == 0:
                oA = ps_o.tile([128, T], FP32, name="oA")
                oB = ps_o.tile([128, T], FP32, name="oB")
            pAB = pp.tile([128, 2, T], BF16, name="pAB")
            nc.scalar.activation(out=pAB, in_=sq.pop(0), func=AF.Exp,
                                 scale=SCALE)
            if u + DEPTH < NU:
                sq.append(emit_scores(u + DEPTH))
            nc.tensor.matmul(
                oA[0:D + 1, :], vg[:, mi, 2 * pair, :], pAB[:, 0, :],
                start=(mi == 0), stop=(mi == MI - 1))
            nc.tensor.matmul(
                oB[0:D + 1, :], vg[:, mi, 2 * pair + 1, :], pAB[:, 1, :],
                start=(mi == 0), stop=(mi == MI - 1))
            if mi == MI - 1:
                # drain PSUM accumulators to SBUF on the (idle) Pool
                # engine so the next pair's AV matmuls reuse the banks
                # without waiting for the normalization chain
                oAc = pr.tile([128, T], FP32, name="oAc")
                nc.vector.tensor_copy(out=oAc[0:D + 1, :],
                                      in_=oA[0:D + 1, :])
                oBc = pr.tile([128, T], FP32, name="oBc")
                nc.vector.tensor_copy(out=oBc[0:D + 1, :],
                                      in_=oB[0:D + 1, :])
                fast = (pair == H // 2 - 1)
                norm_head(oAc, 0, pair, fast)
                norm_head(oBc, 1, pair, fast)

    if STOP_AFTER == "attn":
        return

    # ------------------------------------------------------------------
    # Phase 3+4: per token block: attn projection (fp8 DoubleRow) +
    # residual -> x2[tb] -> LN2 -> h2T[tb]; then x2 += fc2_b
    # ------------------------------------------------------------------
    b2bc = bcast_rows_pool(p_mid, fc2_b, "b2bc")
    with (
        tc.tile_pool(name="pj_work", bufs=3) as pjwork,
        tc.tile_pool(name="ln2_work", bufs=3) as w2,
        tc.tile_pool(name="pj_ps", bufs=2, space="PSUM") as ps_pj,
        tc.tile_pool(name="ln2_ps", bufs=2, space="PSUM") as ps_t2,
    ):
        def emit_ppj(tb):
            ppj = ps_pj.tile([128, 2, 512], FP32, name="ppj")
            for cc in range(2):
                for j in range(4):
                    nc.tensor.matmul(
                        ppj[:, cc, :],
                        oT[:, 2 * j:2 * j + 2, tb * 128:(tb + 1) * 128],
                        wpj[:, 2 * j:2 * j + 2, cc * 512:(cc + 1) * 512],
                        start=(j == 0), stop=(j == 3), perf_mode=DR)
            return ppj

        ppj_q = [emit_ppj(0)]
        for tb in range(TB):
            if tb + 1 < TB:
                ppj_q.append(emit_ppj(tb + 1))
            ppj = ppj_q.pop(0)
            sc = pjwork.tile([128, C], FP32, name="pj_sc")
            _drain(nc, SC_ENG, sc, ppj.rearrange("p a b -> p (a b)"),
                   1.0 / (WS * WS))
            nc.gpsimd.tensor_add(out=x2[:, tb, 0:512], in0=sc[:, 0:512],
                                 in1=xob[:, tb, 0:512])
            nc.gpsimd.tensor_add(out=x2[:, tb, 512:1024],
                                 in0=sc[:, 512:1024],
                                 in1=xob[:, tb, 512:1024])
            # LN2 for this block
            rsig, negmurs = _ln_stats(nc, w2, x2[:, tb, :], eps_sb)
            xn = w2.tile([128, C], BF16, name="ln2_xn")
            nc.scalar.activation(out=xn, in_=x2[:, tb, :], func=AF.Identity,
                                 bias=negmurs, scale=rsig)
            for h4 in range(2):
                tp = ps_t2.tile([128, 4, 128], BF16, name="ln2_tp")
                for j in range(4):
                    cb = h4 * 4 + j
                    nc.tensor.transpose(
                        tp[:, j, :], xn[:, cb * 128:(cb + 1) * 128], ident)
                nc.scalar.activation(
                    out=h2T[:, h4 * 4:(h4 + 1) * 4, tb * 128:(tb + 1) * 128],
                    in_=tp, func=AF.Copy, scale=1.0)
            # fc2 residual base for this block (after LN2 consumed x2[tb])
            nc.gpsimd.tensor_add(out=x2[:, tb, :], in0=x2[:, tb, :],
                                 in1=b2bc)

    pbig_stack.close()
    if STOP_AFTER == "ln2":
        return

    # ------------------------------------------------------------------
    # Phase 5: MLP fc1 (gelu) -> gT [f, t]; fc2 + residual -> y
    # ------------------------------------------------------------------
    with (
        tc.tile_pool(name="p_g", bufs=1) as p_g,
        tc.tile_pool(name="f_w", bufs=2) as fw,
        tc.tile_pool(name="f_out", bufs=4) as fout,
    ):
        gT = p_g.tile([128, FB, T], BF16)
        # stage fc2's second output half in SBUF (first half streams in
        # small tiles hidden under the long fc1+pass-0 compute window)
        w2half = p_g.tile([128, FB, 512], BF16)
        for wq in range(8):
            nc.sync.dma_start(out=w2half[:, wq * 4:(wq + 1) * 4, :],
                              in_=fc2_r[:, wq * 4:(wq + 1) * 4, 512:1024])

        def drain(held, tb, cc):
            yt = fout.tile([128, 512], FP32, name="yt")
            nc.vector.tensor_add(
                out=yt, in0=held[tb],
                in1=x2[:, tb, cc * 512:(cc + 1) * 512])
            nc.sync.dma_start(
                out=y_r[:, tb, cc * 512:(cc + 1) * 512], in_=yt)

        # fc2's first output half (cc=0) accumulates interleaved with fc1
        # (PSUM: 3 fc1 banks + 4 accumulator banks); its drains overlap
        # the second half's accumulation
        with tc.tile_pool(name="f2a_ps", bufs=1, space="PSUM") as ps_f2a:
            held0 = {tb: ps_f2a.tile([128, 512], FP32, name=f"pf2a_{tb}")
                     for tb in range(TB)}
            with tc.tile_pool(name="f1_ps", bufs=3, space="PSUM") as ps_f1:
                for fq in range(FB // 4):
                    w1t = fw.tile([128, CB, 512], BF16, name="w1t")
                    nc.sync.dma_start(
                        out=w1t, in_=fc1_r[:, :, fq * 512:(fq + 1) * 512])
                    for j in range(4):
                        fb = fq * 4 + j
                        pf = ps_f1.tile([128, T], FP32, name="pf")
                        for cb in range(CB):
                            nc.tensor.matmul(
                                pf, w1t[:, cb, j * 128:(j + 1) * 128],
                                h2T[:, cb, :],
                                start=(cb == 0), stop=(cb == CB - 1))
                        nc.scalar.activation(
                            out=gT[:, fb, :], in_=pf, func=AF.Gelu,
                            bias=fc1b_sb[:, fb:fb + 1], scale=1.0)
                    for j in range(4):
                        fb = fq * 4 + j
                        w2t = fw.tile([128, 512], BF16, name="w2t")
                        nc.sync.dma_start(out=w2t,
                                          in_=fc2_r[:, fb, 0:512])
                        for tb in range(TB):
                            nc.tensor.matmul(
                                held0[tb],
                                gT[:, fb, tb * 128:(tb + 1) * 128],
                                w2t,
                                start=(fb == 0), stop=(fb == FB - 1))
            for tb in range(TB):
                drain(held0, tb, 0)

        with tc.tile_pool(name="f2b_ps", bufs=1, space="PSUM") as ps_f2b:
            held1 = {tb: ps_f2b.tile([128, 512], FP32, name=f"pf2b_{tb}")
                     for tb in range(TB)}
            for fb in range(FB):
                for tb in range(TB):
                    nc.tensor.matmul(
                        held1[tb], gT[:, fb, tb * 128:(tb + 1) * 128],
                        w2half[:, fb, :],
                        start=(fb == 0), stop=(fb == FB - 1))
            for tb in range(TB):
                drain(held1, tb, 1)


def split_excess_waits(nc, limit=1):
    """This walrus build only supports ONE sync wait per engine instruction.
    Move excess waits onto NOPs inserted just before the instruction on the
    same engine (for DMAs, move all waits so the descriptor carries none)."""
    for f in nc.m.functions:
        for bb in f.blocks:
            new_insts = []
            for inst in bb.instructions:
                si = getattr(inst, "sync_info", None)
                if si is not None and si.on_wait and len(si.on_wait) > limit:
                    waits = list(si.on_wait)
                    if isinstance(inst, mybir.InstDMACopy):
                        moved, si.on_wait = waits, []
                    else:
                        moved, si.on_wait = waits[limit:], waits[:limit]
                    for j, w in enumerate(moved):
                        nop = mybir.InstNoOp(
                            name=f"{inst.name}-xw{j}",
                            engine=inst.engine,
                            sync_info=mybir.SyncInfo(on_wait=[w], on_update=[]),
                            bass_nofuse=True,
                        )
                        new_insts.append(nop)
                new_insts.append(inst)
            bb.instructions[:] = new_insts


_CACHE = {}


def build():
    key = (SAB_BUFS, STOP_AFTER, K_ENG, V_ENG, Q_ENG, SC_ENG, W1_BUFS, P1H_BUFS)
    if key in _CACHE:
        return _CACHE[key]

    nc = bass.Bass("TRN2", target_bir_lowering=False, debug=False,
                   num_devices=N_CORES)
    io = {}
    io["xb"] = nc.dram_tensor("xb", [M, C], FP32, kind="ExternalInput").ap()
    io["qkv_w"] = nc.dram_tensor("qkv_w", [C, 3 * C], FP8,
                                 kind="ExternalInput").ap()
    io["proj_w"] = nc.dram_tensor("proj_w", [C, C], FP8,
                                  kind="ExternalInput").ap()
    io["proj_b"] = nc.dram_tensor("proj_b", [C], FP32,
                                  kind="ExternalInput").ap()
    io["q_bias"] = nc.dram_tensor("q_bias", [C], FP32,
                                  kind="ExternalInput").ap()
    io["fc1_w"] = nc.dram_tensor("fc1_w", [C, F], BF16,
                                 kind="ExternalInput").ap()
    io["fc1_b"] = nc.dram_tensor("fc1_b", [F], FP32,
                                 kind="ExternalInput").ap()
    io["fc2_w"] = nc.dram_tensor("fc2_w", [F, C], BF16,
                                 kind="ExternalInput").ap()
    io["fc2_b"] = nc.dram_tensor("fc2_b", [C], FP32,
                                 kind="ExternalInput").ap()
    io["y"] = nc.dram_tensor("y", [T, C], FP32, kind="ExternalOutput").ap()

    with tile.TileContext(nc) as tc:
        with ExitStack() as ctx:
            _emit(ctx, tc, io)

    split_excess_waits(nc)
    _CACHE[key] = nc
    return nc


def make_in_maps(inputs):
    import ml_dtypes
    x = np.ascontiguousarray(np.asarray(inputs["x"]), dtype=np.float32)
    f64 = {k: np.asarray(inputs[k], dtype=np.float64)
           for k in ("qkv_w", "proj_w", "proj_b", "ln1_w", "ln1_b", "ln2_w",
                     "ln2_b", "fc1_w", "fc1_b", "fc2_w", "fc2_b")}
    # Fold LayerNorm affines into the weights (exact up to rounding):
    #   h = xn*ln_w + ln_b;  h @ W = xn @ (ln_w[:,None]*W) + ln_b @ W
    # The k-part of the qkv bias cancels in softmax; the v-part commutes
    # through the (row-stochastic) attention matrix into proj_b.
    qkv_eff = f64["qkv_w"] * f64["ln1_w"][:, None]
    qkv_bias = f64["ln1_b"] @ f64["qkv_w"]        # [3072]
    q_bias = qkv_bias[0:C]
    v_bias = qkv_bias[2 * C:3 * C]
    proj_b_eff = f64["proj_b"] + v_bias @ f64["proj_w"]
    fc1_eff = f64["fc1_w"] * f64["ln2_w"][:, None]
    fc1_b_eff = f64["fc1_b"] + f64["ln2_b"] @ f64["fc1_w"]
    weights = {
        "qkv_w": (qkv_eff * WS).astype(np.float32).astype(
            ml_dtypes.float8_e4m3),
        "proj_w": (f64["proj_w"] * WS).astype(np.float32).astype(
            ml_dtypes.float8_e4m3),
        "q_bias": q_bias.astype(np.float32),
        "proj_b": proj_b_eff.astype(np.float32),
        "fc1_w": fc1_eff.astype(np.float32).astype(ml_dtypes.bfloat16),
        "fc1_b": fc1_b_eff.astype(np.float32),
        "fc2_w": f64["fc2_w"].astype(np.float32).astype(ml_dtypes.bfloat16),
        "fc2_b": f64["fc2_b"].astype(np.float32),
    }
    weights = {k: np.ascontiguousarray(v) for k, v in weights.items()}
    maps = []
    for c in range(N_CORES):
        b, q = c // 4, c % 4
        m = dict(weights)
        # rotate so own tokens are rows 0:512 (SPMD-uniform program)
        m["xb"] = np.ascontiguousarray(
            np.roll(x[b], -q * T, axis=0))
        maps.append(m)
    return maps


def assemble(results):
    out = np.empty((B, N, C), dtype=np.float32)
    for c in range(N_CORES):
        b, q = c // 4, c % 4
        out[b, q * T:(q + 1) * T] = results[c]["y"]
    return out


def kernel(**inputs) -> np.ndarray:
    nc = build()
    res = run_bass_kernel_spmd(nc, make_in_maps(inputs), list(range(N_CORES)))
    return assemble(res.results)
